# revision 1
# baseline (speedup 1.0000x reference)
"""AttentionBlock (GroupNorm -> qkv -> 8-head attention -> proj -> residual)
as a distributed Bass/Tile kernel on 8 TRN2 NeuronCores.

Sharding: pure data-parallel over batch B=8 -> one batch element per core,
zero collectives. Each core computes its whole attention block.

Per-core algorithm (C=512, L=1024, NH=8, ch=64, G=32 groups):
  - GroupNorm stats via bn_stats per channel + tiny PE matmuls to reduce
    channel stats to group stats (16 channels/group) and broadcast back.
    gamma/beta are folded into the qkv weights host-side, attention scale
    (ch^-1/4 on q and k) is folded into the q weights as 1/sqrt(ch).
  - qkv as channel matmuls in bf16. q,k produced in natural [c, l] layout;
    v produced directly transposed ([l, c] layout) by swapping matmul
    operands, with the bias added via a K=1 ones-row matmul, so attention
    needs no on-chip transposes at all.
  - scores computed TRANSPOSED: sT[s, t] = k^T q (lhsT=k, rhs=q), softmax
    denominator via an extra ones-column appended to v^T (row 64 of the AV
    output accumulates sum_s P[s, t]).  exp on ScalarE from PSUM -> bf16.
  - AV: a[c, t] = (vT|1)^T @ P accumulated over 8 s-chunks.
  - 1/D via DVE reciprocal_approx_fast, broadcast across partitions with a
    DRAM-bounce DMA, applied while copying AV out of PSUM.
  - proj matmul in bf16 + residual add in f32.
"""

import sys
import types

import numpy as np
import ml_dtypes

BF16 = ml_dtypes.bfloat16

C = 512
L = 1024
NH = 8
CH = 64
G = 32
EPS = 1e-5
N_CORES = 8


# ---------------------------------------------------------------------------
# Environment compat (inlined so kernel.py is self-contained)
# ---------------------------------------------------------------------------
def _install_compat():
    # 1) NTFF profiling hook shim (image's antenv stub lacks axon_hooks).
    try:
        from antenv.axon_hooks import get_axon_ntff_profile_hook  # noqa: F401
    except ImportError:
        try:
            import antenv
            from trn_agent_boot.trn_boot import _ntff_profile_via_ctypes

            m = types.ModuleType("antenv.axon_hooks")
            m._hook = None
            m.set_axon_ntff_profile_hook = lambda h: setattr(m, "_hook", h)
            m.get_axon_ntff_profile_hook = lambda: m._hook
            sys.modules["antenv.axon_hooks"] = m
            antenv.axon_hooks = m
            m.set_axon_ntff_profile_hook(
                _ntff_profile_via_ctypes("/opt/axon/libaxon_pjrt.so")
            )
        except Exception:
            pass

    # 2) gpsimd.sem_clear over a wide semaphore range exceeds this walrus
    #    build's ISA payload limit ("ISA wrong length"); chunk the clears.
    import concourse.bass as bass

    if not getattr(bass.Bass.clear_and_free_semaphores, "_chunk_patch", False):
        _orig_clear = bass.Bass.clear_and_free_semaphores

        def _chunked_clear(self, sems, _orig=_orig_clear):
            sems = list(sems)
            for i in range(0, len(sems), 4):
                _orig(self, sems[i : i + 4])

        _chunked_clear._chunk_patch = True
        bass.Bass.clear_and_free_semaphores = _chunked_clear



def _split_waits(nc):
    """This walrus build accepts at most ONE semaphore wait per instruction;
    Tile emits up to 2 (and the closing drain more). Split the extras into
    standalone EVENT_SEM instructions inserted just before, on the same
    engine, which is semantically identical (same-engine program order)."""
    from concourse import mybir

    nid = 0
    for blk in nc.m.functions[0].blocks:
        new_list = []
        for inst in blk.instructions:
            si = inst.sync_info
            if si and si.on_wait and len(si.on_wait) > 1:
                waits = list(si.on_wait)
                si.on_wait = waits[-1:]
                for w in waits[:-1]:
                    nid += 1
                    ev = mybir.InstEventSemaphore(
                        name=f"WSPLIT-{nid}", ins=[], outs=[]
                    )
                    ev.engine = inst.engine
                    ev.sync_info = mybir.SyncInfo(on_wait=[w], on_update=[])
                    nc.register_instruction(ev, overwrite=True)
                    new_list.append(ev)
            new_list.append(inst)
        blk.instructions[:] = new_list


# ---------------------------------------------------------------------------
# Bass graph
# ---------------------------------------------------------------------------
def build_nc(loop_n=None):
    import concourse.bass as bass
    import concourse.tile as tile
    from concourse import mybir

    f32 = mybir.dt.float32
    bf = mybir.dt.bfloat16
    AF = mybir.ActivationFunctionType
    OP = mybir.AluOpType

    nc = bass.Bass(trn_type="TRN2")
    xd = nc.declare_dram_parameter("x", [C, L], f32, isOutput=False)
    wqkd = nc.declare_dram_parameter("wqk", [C, 2 * C], bf, isOutput=False)
    wvd = nc.declare_dram_parameter("wv", [C, C], bf, isOutput=False)
    wpd = nc.declare_dram_parameter("wp", [C, C], bf, isOutput=False)
    bqkd = nc.declare_dram_parameter("bqk", [128, 8], f32, isOutput=False)
    bvd = nc.declare_dram_parameter("bvb", [128, C], bf, isOutput=False)
    bpd = nc.declare_dram_parameter("bp", [128, 4], f32, isOutput=False)
    indd = nc.declare_dram_parameter("ind", [128, 8], f32, isOutput=False)
    indTd = nc.declare_dram_parameter("indT", [8, 128], f32, isOutput=False)
    outd = nc.declare_dram_parameter("out", [C, L], f32, isOutput=True)

    with tile.TileContext(nc) as tc:
        with (
            tc.tile_pool(name="cst", bufs=1) as cst,
            tc.tile_pool(name="act", bufs=1) as actp,
            tc.tile_pool(name="ptp", bufs=4) as ptp,
            tc.tile_pool(name="dnp", bufs=2) as dnp,
            tc.tile_pool(name="otp", bufs=2) as otp,
            tc.tile_pool(name="psp", bufs=1, space="PSUM") as psp,
            tc.tile_pool(name="drp", bufs=2, space="DRAM") as drp,
        ):
            # ---- load weights/constants
            wqk_sb = cst.tile([128, 4, 2 * C], bf)
            nc.sync.dma_start(
                out=wqk_sb, in_=wqkd[:, :].rearrange("(a p) o -> p a o", p=128)
            )
            wv_sb = cst.tile([128, 4, C], bf)
            nc.sync.dma_start(
                out=wv_sb, in_=wvd[:, :].rearrange("(a p) o -> p a o", p=128)
            )
            wp_sb = cst.tile([128, 4, C], bf)
            nc.sync.dma_start(
                out=wp_sb, in_=wpd[:, :].rearrange("(a p) o -> p a o", p=128)
            )
            bqk_sb = cst.tile([128, 8], f32)
            nc.sync.dma_start(out=bqk_sb, in_=bqkd[:, :])
            bvb_sb = cst.tile([128, C], bf)
            nc.sync.dma_start(out=bvb_sb, in_=bvd[:, :])
            bp_sb = cst.tile([128, 4], f32)
            nc.sync.dma_start(out=bp_sb, in_=bpd[:, :])
            ind_sb = cst.tile([128, 8], f32)
            nc.sync.dma_start(out=ind_sb, in_=indd[:, :])
            indT_sb = cst.tile([8, 128], f32)
            nc.sync.dma_start(out=indT_sb, in_=indTd[:, :])
            eps8 = cst.tile([8, 1], f32)
            nc.vector.memset(eps8, EPS)

            def _emit_body():
                x_sb = actp.tile([128, 4, L], f32)
                xr = xd[:, :].rearrange("(a p) o -> p a o", p=128)
                for t in range(4):
                    nc.sync.dma_start(out=x_sb[:, t, :], in_=xr[:, t, :])

                xn_sb = actp.tile([128, 4, L], bf)
                q_sb = actp.tile([128, 4, L], bf)
                k_sb = actp.tile([128, 4, L], bf)
                vT_sb = actp.tile([128, 8, NH, CH + 1], bf)
                hid_sb = actp.tile([128, 4, L], bf)
                sc_sb = actp.tile([128, 4, 2], f32)

                # ---- GroupNorm statistics
                st6 = actp.tile([128, 4, 2, 6], f32)
                mv = actp.tile([128, 4, 2], f32)
                stats4 = actp.tile([128, 8], f32)
                for t in range(4):
                    for s in range(2):
                        nc.vector.bn_stats(
                            out=st6[:, t, s, :], in_=x_sb[:, t, 512 * s : 512 * (s + 1)]
                        )
                    nc.vector.bn_aggr(out=mv[:, t, :], in_=st6[:, t, :, :])
                    nc.vector.tensor_copy(
                        out=stats4[:, 2 * t : 2 * t + 1], in_=mv[:, t, 0:1]
                    )
                    nc.vector.tensor_mul(
                        out=stats4[:, 2 * t + 1 : 2 * t + 2],
                        in0=mv[:, t, 0:1],
                        in1=mv[:, t, 0:1],
                    )
                    nc.vector.tensor_add(
                        out=stats4[:, 2 * t + 1 : 2 * t + 2],
                        in0=stats4[:, 2 * t + 1 : 2 * t + 2],
                        in1=mv[:, t, 1:2],
                    )
                gmm = psp.tile([8, 8], f32, tag="B", bufs=2)
                nc.tensor.matmul(gmm, lhsT=ind_sb, rhs=stats4, start=True, stop=True)
                gm = actp.tile([8, 8], f32)
                nc.vector.tensor_scalar_mul(out=gm, in0=gmm, scalar1=1.0 / 16.0)
                gmr = gm.rearrange("g (t s) -> g t s", s=2)
                msq = actp.tile([8, 4], f32)
                nc.vector.tensor_mul(out=msq, in0=gmr[:, :, 0], in1=gmr[:, :, 0])
                gv = actp.tile([8, 4], f32)
                nc.vector.tensor_tensor(
                    out=gv, in0=gmr[:, :, 1], in1=msq, op=OP.subtract
                )
                # rsqrt(v + eps) = exp(-0.5 * ln(v + eps)): Log and Exp share one
                # ACT table set (natural_log_exp_and_others), so the softmax Exp
                # later needs no table switch.
                sd = actp.tile([8, 4], f32)
                nc.scalar.activation(out=sd, in_=gv, func=AF.Ln, bias=eps8, scale=1.0)
                inv8 = actp.tile([8, 4], f32)
                nc.scalar.activation(out=inv8, in_=sd, func=AF.Exp, scale=-0.5)
                sh8 = actp.tile([8, 4], f32)
                nc.vector.tensor_mul(out=sh8, in0=gmr[:, :, 0], in1=inv8)
                nc.vector.tensor_scalar_mul(out=sh8, in0=sh8, scalar1=-1.0)
                gs = actp.tile([8, 8], f32)
                gsr = gs.rearrange("g (t s) -> g t s", s=2)
                nc.vector.tensor_copy(out=gsr[:, :, 0], in_=inv8)
                nc.vector.tensor_copy(out=gsr[:, :, 1], in_=sh8)
                for t in range(4):
                    nb = psp.tile([128, 2], f32, tag="B", bufs=2)
                    nc.tensor.matmul(
                        nb, lhsT=indT_sb, rhs=gs[:, 2 * t : 2 * t + 2],
                        start=True, stop=True,
                    )
                    nc.vector.tensor_copy(out=sc_sb[:, t, :], in_=nb)
                for t in range(4):
                    nc.vector.tensor_scalar(
                        out=xn_sb[:, t, :],
                        in0=x_sb[:, t, :],
                        scalar1=sc_sb[:, t, 0:1],
                        scalar2=sc_sb[:, t, 1:2],
                        op0=OP.mult,
                        op1=OP.add,
                    )

                # ---- qkv: q,k in natural [c, l] layout
                for m in range(8):
                    ps = psp.tile([128, L], f32, tag="A", bufs=2)
                    for nh in range(2):
                        for kc in range(4):
                            nc.tensor.matmul(
                                ps[:, 512 * nh : 512 * (nh + 1)],
                                lhsT=wqk_sb[:, kc, 128 * m : 128 * (m + 1)],
                                rhs=xn_sb[:, kc, 512 * nh : 512 * (nh + 1)],
                                start=(kc == 0),
                                stop=(kc == 3),
                            )
                    dst = q_sb if m < 4 else k_sb
                    nc.vector.tensor_scalar_add(
                        out=dst[:, m % 4, :], in0=ps, scalar1=bqk_sb[:, m : m + 1]
                    )

                # ---- v, produced directly transposed: vT[l, c] (+ ones column)
                nc.vector.memset(vT_sb[:, :, :, CH : CH + 1], 1.0)
                for lt in range(8):
                    ps = psp.tile([128, C], f32, tag="B", bufs=2)
                    for kc in range(4):
                        nc.tensor.matmul(
                            ps,
                            lhsT=xn_sb[:, kc, 128 * lt : 128 * (lt + 1)],
                            rhs=wv_sb[:, kc, :],
                            start=(kc == 0),
                            stop=(kc == 3),
                        )
                    nc.vector.tensor_tensor(
                        out=vT_sb[:, lt, :, 0:CH],
                        in0=ps.rearrange("p (h c) -> p h c", h=NH),
                        in1=bvb_sb.rearrange("p (h c) -> p h c", h=NH),
                        op=OP.add,
                    )

                # ---- attention, head pairs (2j at partitions 0:64, 2j+1 at 64:128)
                for j in range(4):
                    m = j
                    pts = [
                        ptp.tile([128, 8, L], bf, tag="pt", name=f"pt{j}_0"),
                        ptp.tile([128, 8, L], bf, tag="pt", name=f"pt{j}_1"),
                    ]
                    for st in range(8):
                        pss = [
                            psp.tile([128, L], f32, tag="A", bufs=2, name=f"qkt{j}_{st}_0"),
                            psp.tile([128, L], f32, tag="A", bufs=2, name=f"qkt{j}_{st}_1"),
                        ]
                        # sequential per-head emission measured faster on HW
                        # than interleaving the pair's matmuls (243us -> 199us)
                        for hh in range(2):
                            for nh in range(2):
                                po = 64 * hh
                                nc.tensor.matmul(
                                    pss[hh][:, 512 * nh : 512 * (nh + 1)],
                                    lhsT=k_sb[po : po + 64, m, 128 * st : 128 * (st + 1)],
                                    rhs=q_sb[po : po + 64, m, 512 * nh : 512 * (nh + 1)],
                                    start=True,
                                    stop=True,
                                )
                        for hh in range(2):
                            nc.scalar.activation(
                                out=pts[hh][:, st, :], in_=pss[hh], func=AF.Exp
                            )
                    for hh in range(2):
                        av = psp.tile([CH + 1, L], f32, tag="B", bufs=2, name=f"av{j}_{hh}")
                        for nh in range(2):
                            for st in range(8):
                                nc.tensor.matmul(
                                    av[:, 512 * nh : 512 * (nh + 1)],
                                    lhsT=vT_sb[:, st, 2 * j + hh, :],
                                    rhs=pts[hh][:, st, 512 * nh : 512 * (nh + 1)],
                                    start=(st == 0),
                                    stop=(st == 7),
                                )
                        dsb = dnp.tile([CH + 1, L], bf, tag="dsb")
                        with nc.allow_low_precision(
                            reason="softmax 1/D in bf16 is within tolerance"
                        ):
                            nc.vector.reciprocal(
                                out=dsb[CH : CH + 1, :], in_=av[CH : CH + 1, :]
                            )
                        ddr = drp.tile([1, L], bf, tag="ddr")
                        nc.sync.dma_start(out=ddr[:, :], in_=dsb[CH : CH + 1, :])
                        dbb = dnp.tile([CH, L], bf, tag="dbb")
                        import concourse.bass as bass_mod

                        bcast = bass_mod.AP(
                            tensor=ddr[:, :].tensor,
                            offset=ddr[:, :].offset,
                            ap=[[0, CH]] + list(ddr[:, :].ap[1:]),
                        )
                        nc.sync.dma_start(out=dbb, in_=bcast)
                        if hh == 0:
                            nc.vector.tensor_mul(
                                out=hid_sb[0:CH, m, :], in0=av[0:CH, :], in1=dbb
                            )
                        else:
                            tmpo = dnp.tile([CH, L], bf, tag="tmpo")
                            nc.vector.tensor_mul(out=tmpo, in0=av[0:CH, :], in1=dbb)
                            nc.sync.dma_start(out=hid_sb[CH:128, m, :], in_=tmpo)

                # ---- residual base: x + b_proj (in place)
                for m in range(4):
                    nc.vector.tensor_scalar_add(
                        out=x_sb[:, m, :], in0=x_sb[:, m, :], scalar1=bp_sb[:, m : m + 1]
                    )

                # ---- proj + residual
                for m in range(4):
                    ps = psp.tile([128, L], f32, tag="A", bufs=2, name=f"proj{m}")
                    for nh in range(2):
                        for kc in range(4):
                            nc.tensor.matmul(
                                ps[:, 512 * nh : 512 * (nh + 1)],
                                lhsT=wp_sb[:, kc, 128 * m : 128 * (m + 1)],
                                rhs=hid_sb[:, kc, 512 * nh : 512 * (nh + 1)],
                                start=(kc == 0),
                                stop=(kc == 3),
                            )
                    ob = otp.tile([128, L], f32, tag="ob")
                    nc.vector.tensor_add(out=ob, in0=ps, in1=x_sb[:, m, :])
                    nc.sync.dma_start(out=outd[128 * m : 128 * (m + 1), :], in_=ob)

            if loop_n:
                with tc.For_i(0, loop_n, 1):
                    _emit_body()
            else:
                _emit_body()

    _split_waits(nc)
    return nc


_NC = None


def _get_nc():
    global _NC
    if _NC is None:
        _install_compat()
        _NC = build_nc()
    return _NC


def _host_prep(x, gamma, beta, w_qkv, b_qkv, w_proj, b_proj):
    x = np.asarray(x, np.float32)
    gamma = np.asarray(gamma, np.float32)
    beta = np.asarray(beta, np.float32)
    w_qkv = np.asarray(w_qkv, np.float32)
    b_qkv = np.asarray(b_qkv, np.float32)
    w_proj = np.asarray(w_proj, np.float32)
    b_proj = np.asarray(b_proj, np.float32)

    s2 = 1.0 / np.sqrt(CH)  # attention scale applied to q AND k => s^2 on q
    Wg = w_qkv * gamma[None, :]
    bb = w_qkv @ beta + b_qkv
    Wg = Wg.copy()
    Wg[0:C] *= s2
    bb = bb.copy()
    bb[0:C] *= s2

    shared = {
        "wqk": np.ascontiguousarray(Wg[0 : 2 * C].T).astype(BF16),
        "wv": np.ascontiguousarray(Wg[2 * C : 3 * C].T).astype(BF16),
        "wp": np.ascontiguousarray(w_proj.T).astype(BF16),
        "bqk": np.ascontiguousarray(bb[0 : 2 * C].reshape(8, 128).T).astype(
            np.float32
        ),
        "bvb": np.broadcast_to(bb[2 * C : 3 * C].reshape(1, C), (128, C)).astype(
            BF16
        ),
        "bp": np.ascontiguousarray(b_proj.reshape(4, 128).T).astype(np.float32),
        "ind": (np.arange(128)[:, None] // 16 == np.arange(8)[None, :]).astype(
            np.float32
        ),
        "indT": (np.arange(128)[None, :] // 16 == np.arange(8)[:, None]).astype(
            np.float32
        ),
    }
    in_maps = []
    for b in range(N_CORES):
        m = dict(shared)
        m["x"] = np.ascontiguousarray(x[b].reshape(C, L))
        in_maps.append(m)
    return in_maps


def run_spmd(in_maps, trace=False):
    from concourse.bass_utils import run_bass_kernel_spmd

    nc = _get_nc()
    return run_bass_kernel_spmd(
        nc, in_maps, core_ids=list(range(N_CORES)), trace=trace
    )


def kernel(x, gamma, beta, w_qkv, b_qkv, w_proj, b_proj):
    _install_compat()
    in_maps = _host_prep(x, gamma, beta, w_qkv, b_qkv, w_proj, b_proj)
    res = run_spmd(in_maps, trace=False)
    out = np.stack(
        [res.results[c]["out"].reshape(C, 32, 32) for c in range(N_CORES)]
    ).astype(np.float32)
    return out



# revision 12
# speedup vs baseline: 1.2528x; 1.2528x over previous
"""AttentionBlock (GroupNorm -> qkv -> 8-head attention -> proj -> residual)
as a distributed Bass/Tile kernel on 8 TRN2 NeuronCores.

Sharding: pure data-parallel over batch B=8 -> one batch element per core,
zero collectives. Each core computes its whole attention block.

Per-core algorithm (C=512, L=1024, NH=8, ch=64, G=32 groups):
  - GroupNorm stats via bn_stats per channel + tiny PE matmuls to reduce
    channel stats to group stats (16 channels/group) and broadcast back.
    gamma/beta are folded into the qkv weights host-side, attention scale
    (ch^-1/4 on q and k) is folded into the q weights as 1/sqrt(ch).
  - qkv as channel matmuls in bf16. q,k produced in natural [c, l] layout;
    v produced directly transposed ([l, c] layout) by swapping matmul
    operands, with the bias added via a K=1 ones-row matmul, so attention
    needs no on-chip transposes at all.
  - scores computed TRANSPOSED: sT[s, t] = k^T q (lhsT=k, rhs=q), softmax
    denominator via an extra ones-column appended to v^T (row 64 of the AV
    output accumulates sum_s P[s, t]).  exp on ScalarE from PSUM -> bf16.
  - AV: a[c, t] = (vT|1)^T @ P accumulated over 8 s-chunks.
  - 1/D via DVE reciprocal_approx_fast, broadcast across partitions with a
    DRAM-bounce DMA, applied while copying AV out of PSUM.
  - proj matmul in bf16 + residual add in f32.
"""

import sys
import types

import numpy as np
import ml_dtypes

BF16 = ml_dtypes.bfloat16

C = 512
L = 1024
NH = 8
CH = 64
G = 32
EPS = 1e-5
N_CORES = 8


# ---------------------------------------------------------------------------
# Environment compat (inlined so kernel.py is self-contained)
# ---------------------------------------------------------------------------
def _install_compat():
    # 1) NTFF profiling hook shim (image's antenv stub lacks axon_hooks).
    try:
        from antenv.axon_hooks import get_axon_ntff_profile_hook  # noqa: F401
    except ImportError:
        try:
            import antenv
            from trn_agent_boot.trn_boot import _ntff_profile_via_ctypes

            m = types.ModuleType("antenv.axon_hooks")
            m._hook = None
            m.set_axon_ntff_profile_hook = lambda h: setattr(m, "_hook", h)
            m.get_axon_ntff_profile_hook = lambda: m._hook
            sys.modules["antenv.axon_hooks"] = m
            antenv.axon_hooks = m
            m.set_axon_ntff_profile_hook(
                _ntff_profile_via_ctypes("/opt/axon/libaxon_pjrt.so")
            )
        except Exception:
            pass

    # 2) gpsimd.sem_clear over a wide semaphore range exceeds this walrus
    #    build's ISA payload limit ("ISA wrong length"); chunk the clears.
    import concourse.bass as bass

    if not getattr(bass.Bass.clear_and_free_semaphores, "_chunk_patch", False):
        _orig_clear = bass.Bass.clear_and_free_semaphores

        def _chunked_clear(self, sems, _orig=_orig_clear):
            sems = list(sems)
            for i in range(0, len(sems), 4):
                _orig(self, sems[i : i + 4])

        _chunked_clear._chunk_patch = True
        bass.Bass.clear_and_free_semaphores = _chunked_clear



def _split_waits(nc):
    """This walrus build accepts at most ONE semaphore wait per instruction;
    Tile emits up to 2 (and the closing drain more). Split the extras into
    standalone EVENT_SEM instructions inserted just before, on the same
    engine, which is semantically identical (same-engine program order)."""
    from concourse import mybir

    nid = 0
    for blk in nc.m.functions[0].blocks:
        new_list = []
        for inst in blk.instructions:
            si = inst.sync_info
            if si and si.on_wait and len(si.on_wait) > 1:
                waits = list(si.on_wait)
                si.on_wait = waits[-1:]
                for w in waits[:-1]:
                    nid += 1
                    ev = mybir.InstEventSemaphore(
                        name=f"WSPLIT-{nid}", ins=[], outs=[]
                    )
                    ev.engine = inst.engine
                    ev.sync_info = mybir.SyncInfo(on_wait=[w], on_update=[])
                    nc.register_instruction(ev, overwrite=True)
                    new_list.append(ev)
            new_list.append(inst)
        blk.instructions[:] = new_list


# ---------------------------------------------------------------------------
# Bass graph
# ---------------------------------------------------------------------------
def build_nc(loop_n=None):
    import concourse.bass as bass
    import concourse.tile as tile
    from concourse import mybir

    f32 = mybir.dt.float32
    bf = mybir.dt.bfloat16
    AF = mybir.ActivationFunctionType
    OP = mybir.AluOpType

    nc = bass.Bass(trn_type="TRN2")
    xd = nc.declare_dram_parameter("x", [C, L], f32, isOutput=False)
    wqkd = nc.declare_dram_parameter("wqk", [C, 2 * C], bf, isOutput=False)
    wvd = nc.declare_dram_parameter("wv", [C, C], bf, isOutput=False)
    wpd = nc.declare_dram_parameter("wp", [C, C], bf, isOutput=False)
    bqkd = nc.declare_dram_parameter("bqk", [128, 8], f32, isOutput=False)
    bvd = nc.declare_dram_parameter("bvb", [128, C], bf, isOutput=False)
    bpd = nc.declare_dram_parameter("bp", [128, 4], f32, isOutput=False)
    indd = nc.declare_dram_parameter("ind", [128, 8], f32, isOutput=False)
    indTd = nc.declare_dram_parameter("indT", [8, 128], f32, isOutput=False)
    outd = nc.declare_dram_parameter("out", [C, L], f32, isOutput=True)

    with tile.TileContext(nc) as tc:
        with (
            tc.tile_pool(name="cst", bufs=1) as cst,
            tc.tile_pool(name="act", bufs=1) as actp,
            tc.tile_pool(name="ptp", bufs=4) as ptp,
            tc.tile_pool(name="dnp", bufs=2) as dnp,
            tc.tile_pool(name="otp", bufs=2) as otp,
            tc.tile_pool(name="psp", bufs=1, space="PSUM") as psp,
            tc.tile_pool(name="drp", bufs=2, space="DRAM") as drp,
        ):
            # ---- weight/constant tiles (DMAs emitted inside the body AFTER
            # the x loads, so GroupNorm isn't queued behind 3MB of weights)
            wqk_sb = cst.tile([128, 4, 2 * C], bf)
            wv_sb = cst.tile([128, 4, C], bf)
            wp_sb = cst.tile([128, 4, C], bf)
            bqk_sb = cst.tile([128, 8], f32)
            bvb_sb = cst.tile([128, C], bf)
            bp_sb = cst.tile([128, 4], f32)
            ind_sb = cst.tile([128, 8], f32)
            indT_sb = cst.tile([8, 128], f32)
            eps8 = cst.tile([8, 1], f32)
            nc.vector.memset(eps8, EPS)

            def _emit_body():
                import concourse.bass as bass_mod

                x_sb = actp.tile([128, 4, L], f32)
                xr = xd[:, :].rearrange("(a p) o -> p a o", p=128)
                for t in range(4):
                    nc.sync.dma_start(out=x_sb[:, t, :], in_=xr[:, t, :])

                # constants, smallest/soonest-needed first
                nc.sync.dma_start(out=ind_sb, in_=indd[:, :])
                nc.sync.dma_start(out=indT_sb, in_=indTd[:, :])
                nc.sync.dma_start(out=bqk_sb, in_=bqkd[:, :])
                nc.sync.dma_start(
                    out=wqk_sb, in_=wqkd[:, :].rearrange("(a p) o -> p a o", p=128)
                )
                nc.sync.dma_start(
                    out=wv_sb, in_=wvd[:, :].rearrange("(a p) o -> p a o", p=128)
                )
                nc.sync.dma_start(out=bvb_sb, in_=bvd[:, :])
                nc.sync.dma_start(
                    out=wp_sb, in_=wpd[:, :].rearrange("(a p) o -> p a o", p=128)
                )
                nc.sync.dma_start(out=bp_sb, in_=bpd[:, :])

                xn_sb = actp.tile([128, 4, L], bf)
                q_sb = actp.tile([128, 4, L], bf)
                k_sb = actp.tile([128, 4, L], bf)
                vT_sb = actp.tile([128, 8, NH, CH + 1], bf)
                hid_sb = actp.tile([128, 4, L], bf)
                sc_sb = actp.tile([128, 4, 2], f32)

                # ---- GroupNorm statistics
                st6 = actp.tile([128, 4, 2, 6], f32)
                mv = actp.tile([128, 4, 2], f32)
                stats4 = actp.tile([128, 8], f32)
                for t in range(4):
                    for s in range(2):
                        nc.vector.bn_stats(
                            out=st6[:, t, s, :], in_=x_sb[:, t, 512 * s : 512 * (s + 1)]
                        )
                    nc.vector.bn_aggr(out=mv[:, t, :], in_=st6[:, t, :, :])
                    nc.vector.tensor_copy(
                        out=stats4[:, 2 * t : 2 * t + 1], in_=mv[:, t, 0:1]
                    )
                    nc.vector.tensor_mul(
                        out=stats4[:, 2 * t + 1 : 2 * t + 2],
                        in0=mv[:, t, 0:1],
                        in1=mv[:, t, 0:1],
                    )
                    nc.vector.tensor_add(
                        out=stats4[:, 2 * t + 1 : 2 * t + 2],
                        in0=stats4[:, 2 * t + 1 : 2 * t + 2],
                        in1=mv[:, t, 1:2],
                    )
                gmm = psp.tile([8, 8], f32, tag="B", bufs=2)
                nc.tensor.matmul(gmm, lhsT=ind_sb, rhs=stats4, start=True, stop=True)
                gm = actp.tile([8, 8], f32)
                nc.vector.tensor_scalar_mul(out=gm, in0=gmm, scalar1=1.0 / 16.0)
                gmr = gm.rearrange("g (t s) -> g t s", s=2)
                msq = actp.tile([8, 4], f32)
                nc.vector.tensor_mul(out=msq, in0=gmr[:, :, 0], in1=gmr[:, :, 0])
                gv = actp.tile([8, 4], f32)
                nc.vector.tensor_tensor(
                    out=gv, in0=gmr[:, :, 1], in1=msq, op=OP.subtract
                )
                # rsqrt(v + eps) = exp(-0.5 * ln(v + eps)): Log and Exp share one
                # ACT table set (natural_log_exp_and_others), so the softmax Exp
                # later needs no table switch.
                sd = actp.tile([8, 4], f32)
                nc.scalar.activation(out=sd, in_=gv, func=AF.Ln, bias=eps8, scale=1.0)
                inv8 = actp.tile([8, 4], f32)
                nc.scalar.activation(out=inv8, in_=sd, func=AF.Exp, scale=-0.5)
                sh8 = actp.tile([8, 4], f32)
                nc.vector.tensor_mul(out=sh8, in0=gmr[:, :, 0], in1=inv8)
                nc.vector.tensor_scalar_mul(out=sh8, in0=sh8, scalar1=-1.0)
                gs = actp.tile([8, 8], f32)
                gsr = gs.rearrange("g (t s) -> g t s", s=2)
                nc.vector.tensor_copy(out=gsr[:, :, 0], in_=inv8)
                nc.vector.tensor_copy(out=gsr[:, :, 1], in_=sh8)
                for t in range(4):
                    nb = psp.tile([128, 2], f32, tag="B", bufs=2)
                    nc.tensor.matmul(
                        nb, lhsT=indT_sb, rhs=gs[:, 2 * t : 2 * t + 2],
                        start=True, stop=True,
                    )
                    nc.vector.tensor_copy(out=sc_sb[:, t, :], in_=nb)
                for t in range(4):
                    nc.vector.tensor_scalar(
                        out=xn_sb[:, t, :],
                        in0=x_sb[:, t, :],
                        scalar1=sc_sb[:, t, 0:1],
                        scalar2=sc_sb[:, t, 1:2],
                        op0=OP.mult,
                        op1=OP.add,
                    )

                # ---- emission helpers for the software pipeline ----------
                def emit_qk_half(m, nh):
                    # half an m-tile (one 512-col nh block) of the q/k matmul;
                    # shares tag-A PSUM slots with the score tiles
                    ps = psp.tile(
                        [128, 512], f32, tag="A", bufs=2, name=f"qkv{m}_{nh}"
                    )
                    for kc in range(4):
                        nc.tensor.matmul(
                            ps,
                            lhsT=wqk_sb[:, kc, 128 * m : 128 * (m + 1)],
                            rhs=xn_sb[:, kc, 512 * nh : 512 * (nh + 1)],
                            start=(kc == 0),
                            stop=(kc == 3),
                            skip_group_check=True,
                        )
                    dst = q_sb if m < 4 else k_sb
                    nc.vector.tensor_scalar_add(
                        out=dst[:, m % 4, 512 * nh : 512 * (nh + 1)],
                        in0=ps,
                        scalar1=bqk_sb[:, m : m + 1],
                    )

                def emit_v_chunk(lt):
                    # vT[l, c] for one 128-l chunk (transposed v via swapped
                    # matmul operands)
                    ps = psp.tile([128, C], f32, tag="A", bufs=2, name=f"v{lt}")
                    for kc in range(4):
                        nc.tensor.matmul(
                            ps,
                            lhsT=xn_sb[:, kc, 128 * lt : 128 * (lt + 1)],
                            rhs=wv_sb[:, kc, :],
                            start=(kc == 0),
                            stop=(kc == 3),
                            skip_group_check=True,
                        )
                    nc.vector.tensor_tensor(
                        out=vT_sb[:, lt, :, 0:CH],
                        in0=ps.rearrange("p (h c) -> p h c", h=NH),
                        in1=bvb_sb.rearrange("p (h c) -> p h c", h=NH),
                        op=OP.add,
                    )

                pts_t = {}
                av_t = {}

                def emit_qk_chunk(j, st):
                    # scores^T [s-chunk, t] for head pair j + exp on ScalarE
                    pss = [
                        psp.tile([128, L], f32, tag="A", bufs=2, name=f"qkt{j}_{st}_0"),
                        psp.tile([128, L], f32, tag="A", bufs=2, name=f"qkt{j}_{st}_1"),
                    ]
                    for hh in range(2):
                        po = 64 * hh
                        for nh in range(2):
                            nc.tensor.matmul(
                                pss[hh][:, 512 * nh : 512 * (nh + 1)],
                                lhsT=k_sb[po : po + 64, j, 128 * st : 128 * (st + 1)],
                                rhs=q_sb[po : po + 64, j, 512 * nh : 512 * (nh + 1)],
                                start=True,
                                stop=True,
                                skip_group_check=True,
                            )
                    for hh in range(2):
                        nc.scalar.activation(
                            out=pts_t[j][hh][:, st, :], in_=pss[hh], func=AF.Exp
                        )

                def emit_av_chunk(j, st):
                    # accumulate (vT|1)^T @ P for both heads of pair j
                    for hh in range(2):
                        for nh in range(2):
                            nc.tensor.matmul(
                                av_t[j][hh][:, 512 * nh : 512 * (nh + 1)],
                                lhsT=vT_sb[:, st, 2 * j + hh, :],
                                rhs=pts_t[j][hh][:, st, 512 * nh : 512 * (nh + 1)],
                                start=(st == 0),
                                stop=(st == 7),
                                skip_group_check=True,
                            )

                def emit_norm(j):
                    # 1/D: DVE's iterative-divide reciprocal costs ~6.1
                    # cyc/elem PER LANE (free-size bound), so a [1, 1024] row
                    # costs 6.5us. DMA cannot read PSUM, so DVE-copy both
                    # heads' D rows into one [2, L] SBUF tile, retile it to
                    # [16, 128] via an SBUF->SBUF DMA (512B runs) so 16 lanes
                    # share the reciprocal (~0.8us), then bounce through DRAM
                    # for the partition broadcast.
                    dT = dnp.tile([16, 128], f32, tag="dT")
                    for hh in range(2):
                        dsb = dnp.tile([1, L], f32, tag=f"dsb{hh}")
                        nc.vector.tensor_copy(
                            out=dsb, in_=av_t[j][hh][CH : CH + 1, :]
                        )
                        # NB: keep the source partition dim at 1 — splitting
                        # the free dim into the AP's partition slot would read
                        # physical partitions 1..7 (other tiles), not the row.
                        nc.sync.dma_start(
                            out=dT[8 * hh : 8 * (hh + 1), :],
                            in_=dsb.rearrange("o (p a) -> o p a", p=8),
                        )
                    rT = dnp.tile([16, 128], f32, tag="rT")
                    nc.vector.reciprocal(out=rT, in_=dT)
                    ddr = drp.tile([2, L], f32, tag="ddr")
                    nc.sync.dma_start(
                        out=ddr[:, :].rearrange("h (p a) -> (h p) a", p=8),
                        in_=rT,
                    )
                    for hh in range(2):
                        av = av_t[j][hh]
                        dbb = dnp.tile([CH, L], f32, tag="dbb")
                        row = ddr[hh : hh + 1, :]
                        bcast = bass_mod.AP(
                            tensor=row.tensor,
                            offset=row.offset,
                            ap=[[0, CH]] + list(row.ap[1:]),
                        )
                        nc.sync.dma_start(out=dbb, in_=bcast)
                        if hh == 0:
                            nc.vector.tensor_mul(
                                out=hid_sb[0:CH, j, :], in0=av[0:CH, :], in1=dbb
                            )
                        else:
                            tmpo = dnp.tile([CH, L], bf, tag="tmpo")
                            nc.vector.tensor_mul(out=tmpo, in0=av[0:CH, :], in1=dbb)
                            nc.sync.dma_start(out=hid_sb[CH:128, j, :], in_=tmpo)

                def alloc_pts(j):
                    pts_t[j] = [
                        ptp.tile([128, 8, L], bf, tag="pt", name=f"pt{j}_0"),
                        ptp.tile([128, 8, L], bf, tag="pt", name=f"pt{j}_1"),
                    ]

                def alloc_av(j):
                    av_t[j] = [
                        psp.tile([CH + 1, L], f32, tag="B", bufs=2, name=f"av{j}_0"),
                        psp.tile([CH + 1, L], f32, tag="B", bufs=2, name=f"av{j}_1"),
                    ]

                # ---- software pipeline --------------------------------------
                # PE order: q/k for pair 0 first so exp (the ScalarE
                # bottleneck) starts ASAP; v chunks + later q/k pairs + the
                # previous pair's AV fill PE slack inside each st loop, so
                # ScalarE never starves and PE never idles long.
                nc.vector.memset(vT_sb[:, :, :, CH : CH + 1], 1.0)
                for nh in range(2):
                    emit_qk_half(0, nh)
                for nh in range(2):
                    emit_qk_half(4, nh)

                # fillers[j][st] -> list of zero-arg emitters
                def half(m, nh):
                    return lambda: emit_qk_half(m, nh)

                def vch(lt):
                    return lambda: emit_v_chunk(lt)

                fillers = {
                    0: {
                        1: [vch(0)], 2: [vch(1)], 3: [vch(2)],
                        4: [vch(3), half(1, 0)], 5: [vch(4), half(1, 1)],
                        6: [vch(5), half(5, 0)], 7: [vch(6), half(5, 1)],
                    },
                    1: {
                        0: [vch(7)],
                        4: [half(2, 0)], 5: [half(2, 1)],
                        6: [half(6, 0)], 7: [half(6, 1)],
                    },
                    2: {
                        4: [half(3, 0)], 5: [half(3, 1)],
                        6: [half(7, 0)], 7: [half(7, 1)],
                    },
                    3: {},
                }

                for j in range(4):
                    alloc_pts(j)
                    if j > 0:
                        alloc_av(j - 1)
                    for st in range(8):
                        emit_qk_chunk(j, st)
                        if j > 0:
                            emit_av_chunk(j - 1, st)
                        for f in fillers[j].get(st, []):
                            f()
                    if j > 0:
                        emit_norm(j - 1)
                    if j == 0:
                        # residual base: x + b_proj (in place), off the
                        # critical path while ScalarE grinds exps
                        for m in range(4):
                            nc.vector.tensor_scalar_add(
                                out=x_sb[:, m, :],
                                in0=x_sb[:, m, :],
                                scalar1=bp_sb[:, m : m + 1],
                            )
                alloc_av(3)
                for st in range(8):
                    emit_av_chunk(3, st)
                emit_norm(3)

                # ---- proj + residual
                for m in range(4):
                    ps = psp.tile([128, L], f32, tag="A", bufs=2, name=f"proj{m}")
                    for nh in range(2):
                        for kc in range(4):
                            nc.tensor.matmul(
                                ps[:, 512 * nh : 512 * (nh + 1)],
                                lhsT=wp_sb[:, kc, 128 * m : 128 * (m + 1)],
                                rhs=hid_sb[:, kc, 512 * nh : 512 * (nh + 1)],
                                start=(kc == 0),
                                stop=(kc == 3),
                                skip_group_check=True,
                            )
                    ob = otp.tile([128, L], f32, tag="ob")
                    nc.vector.tensor_add(out=ob, in0=ps, in1=x_sb[:, m, :])
                    nc.sync.dma_start(out=outd[128 * m : 128 * (m + 1), :], in_=ob)

            if loop_n:
                with tc.For_i(0, loop_n, 1):
                    _emit_body()
            else:
                _emit_body()

    _split_waits(nc)
    return nc


_NC = None


def _get_nc():
    global _NC
    if _NC is None:
        _install_compat()
        _NC = build_nc()
    return _NC


def _host_prep(x, gamma, beta, w_qkv, b_qkv, w_proj, b_proj):
    x = np.asarray(x, np.float32)
    gamma = np.asarray(gamma, np.float32)
    beta = np.asarray(beta, np.float32)
    w_qkv = np.asarray(w_qkv, np.float32)
    b_qkv = np.asarray(b_qkv, np.float32)
    w_proj = np.asarray(w_proj, np.float32)
    b_proj = np.asarray(b_proj, np.float32)

    s2 = 1.0 / np.sqrt(CH)  # attention scale applied to q AND k => s^2 on q
    Wg = w_qkv * gamma[None, :]
    bb = w_qkv @ beta + b_qkv
    Wg = Wg.copy()
    Wg[0:C] *= s2
    bb = bb.copy()
    bb[0:C] *= s2

    shared = {
        "wqk": np.ascontiguousarray(Wg[0 : 2 * C].T).astype(BF16),
        "wv": np.ascontiguousarray(Wg[2 * C : 3 * C].T).astype(BF16),
        "wp": np.ascontiguousarray(w_proj.T).astype(BF16),
        "bqk": np.ascontiguousarray(bb[0 : 2 * C].reshape(8, 128).T).astype(
            np.float32
        ),
        "bvb": np.broadcast_to(bb[2 * C : 3 * C].reshape(1, C), (128, C)).astype(
            BF16
        ),
        "bp": np.ascontiguousarray(b_proj.reshape(4, 128).T).astype(np.float32),
        "ind": (np.arange(128)[:, None] // 16 == np.arange(8)[None, :]).astype(
            np.float32
        ),
        "indT": (np.arange(128)[None, :] // 16 == np.arange(8)[:, None]).astype(
            np.float32
        ),
    }
    in_maps = []
    for b in range(N_CORES):
        m = dict(shared)
        m["x"] = np.ascontiguousarray(x[b].reshape(C, L))
        in_maps.append(m)
    return in_maps


def run_spmd(in_maps, trace=False):
    from concourse.bass_utils import run_bass_kernel_spmd

    nc = _get_nc()
    return run_bass_kernel_spmd(
        nc, in_maps, core_ids=list(range(N_CORES)), trace=trace
    )


def kernel(x, gamma, beta, w_qkv, b_qkv, w_proj, b_proj):
    _install_compat()
    in_maps = _host_prep(x, gamma, beta, w_qkv, b_qkv, w_proj, b_proj)
    res = run_spmd(in_maps, trace=False)
    out = np.stack(
        [res.results[c]["out"].reshape(C, 32, 32) for c in range(N_CORES)]
    ).astype(np.float32)
    return out



# revision 15
# speedup vs baseline: 1.4413x; 1.1504x over previous
"""AttentionBlock (GroupNorm -> qkv -> 8-head attention -> proj -> residual)
as a distributed Bass/Tile kernel on 8 TRN2 NeuronCores.

Sharding: pure data-parallel over batch B=8 -> one batch element per core,
zero collectives. Each core computes its whole attention block.

Per-core algorithm (C=512, L=1024, NH=8, ch=64, G=32 groups):
  - GroupNorm stats via bn_stats per channel + tiny PE matmuls to reduce
    channel stats to group stats (16 channels/group) and broadcast back.
    gamma/beta are folded into the qkv weights host-side, attention scale
    (ch^-1/4 on q and k) is folded into the q weights as 1/sqrt(ch).
  - qkv as channel matmuls in bf16. q,k produced in natural [c, l] layout;
    v produced directly transposed ([l, c] layout) by swapping matmul
    operands, with the bias added via a K=1 ones-row matmul, so attention
    needs no on-chip transposes at all.
  - scores computed TRANSPOSED: sT[s, t] = k^T q (lhsT=k, rhs=q), softmax
    denominator via an extra ones-column appended to v^T (row 64 of the AV
    output accumulates sum_s P[s, t]).  exp on ScalarE from PSUM -> bf16.
  - AV: a[c, t] = (vT|1)^T @ P accumulated over 8 s-chunks.
  - 1/D via DVE reciprocal_approx_fast, broadcast across partitions with a
    DRAM-bounce DMA, applied while copying AV out of PSUM.
  - proj matmul in bf16 + residual add in f32.
"""

import sys
import types

import numpy as np
import ml_dtypes

BF16 = ml_dtypes.bfloat16

C = 512
L = 1024
NH = 8
CH = 64
G = 32
EPS = 1e-5
N_CORES = 8


# ---------------------------------------------------------------------------
# Environment compat (inlined so kernel.py is self-contained)
# ---------------------------------------------------------------------------
def _install_compat():
    # 1) NTFF profiling hook shim (image's antenv stub lacks axon_hooks).
    try:
        from antenv.axon_hooks import get_axon_ntff_profile_hook  # noqa: F401
    except ImportError:
        try:
            import antenv
            from trn_agent_boot.trn_boot import _ntff_profile_via_ctypes

            m = types.ModuleType("antenv.axon_hooks")
            m._hook = None
            m.set_axon_ntff_profile_hook = lambda h: setattr(m, "_hook", h)
            m.get_axon_ntff_profile_hook = lambda: m._hook
            sys.modules["antenv.axon_hooks"] = m
            antenv.axon_hooks = m
            m.set_axon_ntff_profile_hook(
                _ntff_profile_via_ctypes("/opt/axon/libaxon_pjrt.so")
            )
        except Exception:
            pass

    # 2) gpsimd.sem_clear over a wide semaphore range exceeds this walrus
    #    build's ISA payload limit ("ISA wrong length"); chunk the clears.
    import concourse.bass as bass

    if not getattr(bass.Bass.clear_and_free_semaphores, "_chunk_patch", False):
        _orig_clear = bass.Bass.clear_and_free_semaphores

        def _chunked_clear(self, sems, _orig=_orig_clear):
            sems = list(sems)
            for i in range(0, len(sems), 4):
                _orig(self, sems[i : i + 4])

        _chunked_clear._chunk_patch = True
        bass.Bass.clear_and_free_semaphores = _chunked_clear



def _split_waits(nc):
    """This walrus build accepts at most ONE semaphore wait per instruction;
    Tile emits up to 2 (and the closing drain more). Split the extras into
    standalone EVENT_SEM instructions inserted just before, on the same
    engine, which is semantically identical (same-engine program order)."""
    from concourse import mybir

    nid = 0
    for blk in nc.m.functions[0].blocks:
        new_list = []
        for inst in blk.instructions:
            si = inst.sync_info
            if si and si.on_wait and len(si.on_wait) > 1:
                waits = list(si.on_wait)
                si.on_wait = waits[-1:]
                for w in waits[:-1]:
                    nid += 1
                    ev = mybir.InstEventSemaphore(
                        name=f"WSPLIT-{nid}", ins=[], outs=[]
                    )
                    ev.engine = inst.engine
                    ev.sync_info = mybir.SyncInfo(on_wait=[w], on_update=[])
                    nc.register_instruction(ev, overwrite=True)
                    new_list.append(ev)
            new_list.append(inst)
        blk.instructions[:] = new_list


# ---------------------------------------------------------------------------
# Bass graph
# ---------------------------------------------------------------------------
def build_nc(loop_n=None):
    import concourse.bass as bass
    import concourse.tile as tile
    from concourse import mybir

    f32 = mybir.dt.float32
    bf = mybir.dt.bfloat16
    AF = mybir.ActivationFunctionType
    OP = mybir.AluOpType

    nc = bass.Bass(trn_type="TRN2")
    xd = nc.declare_dram_parameter("x", [C, L], f32, isOutput=False)
    wqkd = nc.declare_dram_parameter("wqk", [C, 2 * C], bf, isOutput=False)
    wvd = nc.declare_dram_parameter("wv", [C, C], bf, isOutput=False)
    wpd = nc.declare_dram_parameter("wp", [C, C], bf, isOutput=False)
    bqkd = nc.declare_dram_parameter("bqk", [128, 8], f32, isOutput=False)
    bvd = nc.declare_dram_parameter("bvb", [128, C], bf, isOutput=False)
    bpd = nc.declare_dram_parameter("bp", [128, 4], f32, isOutput=False)
    indd = nc.declare_dram_parameter("ind", [128, 8], f32, isOutput=False)
    indTd = nc.declare_dram_parameter("indT", [8, 128], f32, isOutput=False)
    outd = nc.declare_dram_parameter("out", [C, L], f32, isOutput=True)

    with tile.TileContext(nc) as tc:
        with (
            tc.tile_pool(name="cst", bufs=1) as cst,
            tc.tile_pool(name="act", bufs=1) as actp,
            tc.tile_pool(name="ptp", bufs=4) as ptp,
            tc.tile_pool(name="dnp", bufs=2) as dnp,
            tc.tile_pool(name="otp", bufs=2) as otp,
            tc.tile_pool(name="psp", bufs=1, space="PSUM") as psp,
            tc.tile_pool(name="drp", bufs=2, space="DRAM") as drp,
        ):
            # ---- weight/constant tiles (DMAs emitted inside the body AFTER
            # the x loads, so GroupNorm isn't queued behind 3MB of weights)
            wqk_sb = cst.tile([128, 4, 2 * C], bf)
            wv_sb = cst.tile([128, 4, C], bf)
            wp_sb = cst.tile([128, 4, C], bf)
            bqk_sb = cst.tile([128, 8], f32)
            bvb_sb = cst.tile([128, C], bf)
            bp_sb = cst.tile([128, 4], f32)
            ind_sb = cst.tile([128, 8], f32)
            indT_sb = cst.tile([8, 128], f32)
            eps8 = cst.tile([8, 1], f32)
            nc.vector.memset(eps8, EPS)

            def _emit_body():
                import concourse.bass as bass_mod

                x_sb = actp.tile([128, 4, L], f32)
                xr = xd[:, :].rearrange("(a p) o -> p a o", p=128)
                for t in range(4):
                    nc.sync.dma_start(out=x_sb[:, t, :], in_=xr[:, t, :])

                # constants, smallest/soonest-needed first
                nc.sync.dma_start(out=ind_sb, in_=indd[:, :])
                nc.sync.dma_start(out=indT_sb, in_=indTd[:, :])
                nc.sync.dma_start(out=bqk_sb, in_=bqkd[:, :])
                nc.sync.dma_start(
                    out=wqk_sb, in_=wqkd[:, :].rearrange("(a p) o -> p a o", p=128)
                )
                nc.sync.dma_start(
                    out=wv_sb, in_=wvd[:, :].rearrange("(a p) o -> p a o", p=128)
                )
                nc.sync.dma_start(out=bvb_sb, in_=bvd[:, :])
                nc.sync.dma_start(
                    out=wp_sb, in_=wpd[:, :].rearrange("(a p) o -> p a o", p=128)
                )
                nc.sync.dma_start(out=bp_sb, in_=bpd[:, :])

                xn_sb = actp.tile([128, 4, L], bf)
                q_sb = actp.tile([128, 4, L], bf)
                k_sb = actp.tile([128, 4, L], bf)
                vT_sb = actp.tile([128, 8, NH, CH + 1], bf)
                hid_sb = actp.tile([128, 4, L], bf)
                sc_sb = actp.tile([128, 4, 2], f32)

                # ---- GroupNorm statistics
                st6 = actp.tile([128, 4, 2, 6], f32)
                mv = actp.tile([128, 4, 2], f32)
                stats4 = actp.tile([128, 8], f32)
                for t in range(4):
                    for s in range(2):
                        nc.vector.bn_stats(
                            out=st6[:, t, s, :], in_=x_sb[:, t, 512 * s : 512 * (s + 1)]
                        )
                    nc.vector.bn_aggr(out=mv[:, t, :], in_=st6[:, t, :, :])
                    nc.vector.tensor_copy(
                        out=stats4[:, 2 * t : 2 * t + 1], in_=mv[:, t, 0:1]
                    )
                    nc.vector.tensor_mul(
                        out=stats4[:, 2 * t + 1 : 2 * t + 2],
                        in0=mv[:, t, 0:1],
                        in1=mv[:, t, 0:1],
                    )
                    nc.vector.tensor_add(
                        out=stats4[:, 2 * t + 1 : 2 * t + 2],
                        in0=stats4[:, 2 * t + 1 : 2 * t + 2],
                        in1=mv[:, t, 1:2],
                    )
                gmm = psp.tile([8, 8], f32, tag="B", bufs=2)
                nc.tensor.matmul(gmm, lhsT=ind_sb, rhs=stats4, start=True, stop=True)
                gm = actp.tile([8, 8], f32)
                nc.vector.tensor_scalar_mul(out=gm, in0=gmm, scalar1=1.0 / 16.0)
                gmr = gm.rearrange("g (t s) -> g t s", s=2)
                msq = actp.tile([8, 4], f32)
                nc.vector.tensor_mul(out=msq, in0=gmr[:, :, 0], in1=gmr[:, :, 0])
                gv = actp.tile([8, 4], f32)
                nc.vector.tensor_tensor(
                    out=gv, in0=gmr[:, :, 1], in1=msq, op=OP.subtract
                )
                # rsqrt(v + eps) = exp(-0.5 * ln(v + eps)): Log and Exp share one
                # ACT table set (natural_log_exp_and_others), so the softmax Exp
                # later needs no table switch.
                sd = actp.tile([8, 4], f32)
                nc.scalar.activation(out=sd, in_=gv, func=AF.Ln, bias=eps8, scale=1.0)
                inv8 = actp.tile([8, 4], f32)
                nc.scalar.activation(out=inv8, in_=sd, func=AF.Exp, scale=-0.5)
                sh8 = actp.tile([8, 4], f32)
                nc.vector.tensor_mul(out=sh8, in0=gmr[:, :, 0], in1=inv8)
                nc.vector.tensor_scalar_mul(out=sh8, in0=sh8, scalar1=-1.0)
                gs = actp.tile([8, 8], f32)
                gsr = gs.rearrange("g (t s) -> g t s", s=2)
                nc.vector.tensor_copy(out=gsr[:, :, 0], in_=inv8)
                nc.vector.tensor_copy(out=gsr[:, :, 1], in_=sh8)
                nb = psp.tile([128, 8], f32, tag="B", bufs=2)
                nc.tensor.matmul(nb, lhsT=indT_sb, rhs=gs, start=True, stop=True)
                nc.vector.tensor_copy(
                    out=sc_sb.rearrange("p t s -> p (t s)"), in_=nb
                )
                for t in range(4):
                    nc.vector.tensor_scalar(
                        out=xn_sb[:, t, :],
                        in0=x_sb[:, t, :],
                        scalar1=sc_sb[:, t, 0:1],
                        scalar2=sc_sb[:, t, 1:2],
                        op0=OP.mult,
                        op1=OP.add,
                    )

                # ---- emission helpers for the software pipeline ----------
                def emit_qk_half(m, nh):
                    # half an m-tile (one 512-col nh block) of the q/k matmul;
                    # shares tag-A PSUM slots with the score tiles
                    ps = psp.tile(
                        [128, 512], f32, tag="A", bufs=2, name=f"qkv{m}_{nh}"
                    )
                    for kc in range(4):
                        nc.tensor.matmul(
                            ps,
                            lhsT=wqk_sb[:, kc, 128 * m : 128 * (m + 1)],
                            rhs=xn_sb[:, kc, 512 * nh : 512 * (nh + 1)],
                            start=(kc == 0),
                            stop=(kc == 3),
                            skip_group_check=True,
                        )
                    dst = q_sb if m < 4 else k_sb
                    nc.vector.tensor_scalar_add(
                        out=dst[:, m % 4, 512 * nh : 512 * (nh + 1)],
                        in0=ps,
                        scalar1=bqk_sb[:, m : m + 1],
                    )

                def emit_v_chunk(lt):
                    # vT[l, c] for one 128-l chunk (transposed v via swapped
                    # matmul operands)
                    ps = psp.tile([128, C], f32, tag="A", bufs=2, name=f"v{lt}")
                    for kc in range(4):
                        nc.tensor.matmul(
                            ps,
                            lhsT=xn_sb[:, kc, 128 * lt : 128 * (lt + 1)],
                            rhs=wv_sb[:, kc, :],
                            start=(kc == 0),
                            stop=(kc == 3),
                            skip_group_check=True,
                        )
                    nc.vector.tensor_tensor(
                        out=vT_sb[:, lt, :, 0:CH],
                        in0=ps.rearrange("p (h c) -> p h c", h=NH),
                        in1=bvb_sb.rearrange("p (h c) -> p h c", h=NH),
                        op=OP.add,
                    )

                pts_t = {}
                av_t = {}

                def emit_qk_chunk(j, st):
                    # scores^T [s-chunk, t] for head pair j + exp on ScalarE
                    pss = [
                        psp.tile([128, L], f32, tag="A", bufs=2, name=f"qkt{j}_{st}_0"),
                        psp.tile([128, L], f32, tag="A", bufs=2, name=f"qkt{j}_{st}_1"),
                    ]
                    for hh in range(2):
                        po = 64 * hh
                        for nh in range(2):
                            nc.tensor.matmul(
                                pss[hh][:, 512 * nh : 512 * (nh + 1)],
                                lhsT=k_sb[po : po + 64, j, 128 * st : 128 * (st + 1)],
                                rhs=q_sb[po : po + 64, j, 512 * nh : 512 * (nh + 1)],
                                start=True,
                                stop=True,
                                skip_group_check=True,
                            )
                    for hh in range(2):
                        nc.scalar.activation(
                            out=pts_t[j][hh][:, st, :], in_=pss[hh], func=AF.Exp
                        )

                def emit_av_chunk(j, st):
                    # accumulate (vT|1)^T @ P for both heads of pair j
                    for hh in range(2):
                        for nh in range(2):
                            nc.tensor.matmul(
                                av_t[j][hh][:, 512 * nh : 512 * (nh + 1)],
                                lhsT=vT_sb[:, st, 2 * j + hh, :],
                                rhs=pts_t[j][hh][:, st, 512 * nh : 512 * (nh + 1)],
                                start=(st == 0),
                                stop=(st == 7),
                                skip_group_check=True,
                            )

                def emit_norm(j):
                    # Softmax normalization. First copy each head's AV (incl.
                    # the D row) out of PSUM into bf16 SBUF — this frees the
                    # PSUM B slots immediately so the next pair's AV can
                    # accumulate while the 1/D chain below is in flight.
                    ah = []
                    for hh in range(2):
                        a = dnp.tile([CH + 1, L], bf, tag=f"ah{hh}")
                        nc.vector.tensor_copy(out=a, in_=av_t[j][hh])
                        ah.append(a)
                    # DVE's iterative-divide reciprocal costs ~6 cyc/elem PER
                    # LANE (free-size bound): [1, 1024] would be 6.5us. Retile
                    # both D rows to [16, 128] via SBUF->SBUF DMA so 16 lanes
                    # share the work (~0.8us). NB: keep the source partition
                    # dim at 1 — splitting the free dim into the AP's
                    # partition slot would read physical partitions 1..7.
                    dT = dnp.tile([16, 128], bf, tag="dT")
                    for hh in range(2):
                        nc.sync.dma_start(
                            out=dT[8 * hh : 8 * (hh + 1), :],
                            in_=ah[hh][CH : CH + 1, :].rearrange(
                                "o (p a) -> o p a", p=8
                            ),
                        )
                    rT = dnp.tile([16, 128], bf, tag="rT")
                    with nc.allow_low_precision(
                        reason="softmax 1/D in bf16 is within tolerance"
                    ):
                        nc.vector.reciprocal(out=rT, in_=dT)
                    ddr = drp.tile([2, L], bf, tag="ddr")
                    nc.sync.dma_start(
                        out=ddr[:, :].rearrange("h (p a) -> (h p) a", p=8),
                        in_=rT,
                    )
                    for hh in (1, 0):
                        dbb = dnp.tile([CH, L], bf, tag=f"dbb{hh}")
                        row = ddr[hh : hh + 1, :]
                        bcast = bass_mod.AP(
                            tensor=row.tensor,
                            offset=row.offset,
                            ap=[[0, CH]] + list(row.ap[1:]),
                        )
                        nc.sync.dma_start(out=dbb, in_=bcast)
                        if hh == 0:
                            nc.vector.tensor_mul(
                                out=hid_sb[0:CH, j, :], in0=ah[0][0:CH, :], in1=dbb
                            )
                        else:
                            tmpo = dnp.tile([CH, L], bf, tag="tmpo")
                            nc.vector.tensor_mul(
                                out=tmpo, in0=ah[1][0:CH, :], in1=dbb
                            )
                            nc.sync.dma_start(out=hid_sb[CH:128, j, :], in_=tmpo)

                def alloc_pts(j):
                    pts_t[j] = [
                        ptp.tile([128, 8, L], bf, tag="pt", name=f"pt{j}_0"),
                        ptp.tile([128, 8, L], bf, tag="pt", name=f"pt{j}_1"),
                    ]

                def alloc_av(j):
                    av_t[j] = [
                        psp.tile([CH + 1, L], f32, tag="B", bufs=2, name=f"av{j}_0"),
                        psp.tile([CH + 1, L], f32, tag="B", bufs=2, name=f"av{j}_1"),
                    ]

                # ---- software pipeline --------------------------------------
                # PE order: q/k for pair 0 first so exp (the ScalarE
                # bottleneck) starts ASAP; v chunks + later q/k pairs + the
                # previous pair's AV fill PE slack inside each st loop, so
                # ScalarE never starves and PE never idles long.
                nc.vector.memset(vT_sb[:, :, :, CH : CH + 1], 1.0)
                for nh in range(2):
                    emit_qk_half(0, nh)
                for nh in range(2):
                    emit_qk_half(4, nh)

                # fillers[j][st] -> list of zero-arg emitters
                def half(m, nh):
                    return lambda: emit_qk_half(m, nh)

                def vch(lt):
                    return lambda: emit_v_chunk(lt)

                # v chunk lt must be emitted before AV(j=0, lt); q/k halves
                # for pair j+1 must be emitted before QK(j+1, 0)
                fillers = {
                    0: {
                        0: [vch(0)], 1: [vch(1)], 2: [vch(2)], 3: [vch(3)],
                        4: [vch(4)], 5: [vch(5), half(1, 0)],
                        6: [vch(6), half(1, 1)], 7: [vch(7), half(5, 0)],
                    },
                    1: {
                        4: [half(2, 0)], 5: [half(2, 1)],
                        6: [half(6, 0)], 7: [half(6, 1)],
                    },
                    2: {
                        4: [half(3, 0)], 5: [half(3, 1)],
                        6: [half(7, 0)], 7: [half(7, 1)],
                    },
                    3: {},
                }

                for j in range(4):
                    alloc_pts(j)
                    alloc_av(j)
                    if j == 1:
                        emit_qk_half(5, 1)
                    for st in range(8):
                        emit_qk_chunk(j, st)
                        if st > 0:
                            emit_av_chunk(j, st - 1)
                        for f in fillers[j].get(st, []):
                            f()
                    emit_av_chunk(j, 7)
                    emit_norm(j)
                    if j == 0:
                        # residual base: x + b_proj (in place), off the
                        # critical path while ScalarE grinds exps
                        for m in range(4):
                            nc.vector.tensor_scalar_add(
                                out=x_sb[:, m, :],
                                in0=x_sb[:, m, :],
                                scalar1=bp_sb[:, m : m + 1],
                            )

                # ---- proj + residual
                for m in range(4):
                    ps = psp.tile([128, L], f32, tag="A", bufs=2, name=f"proj{m}")
                    for nh in range(2):
                        for kc in range(4):
                            nc.tensor.matmul(
                                ps[:, 512 * nh : 512 * (nh + 1)],
                                lhsT=wp_sb[:, kc, 128 * m : 128 * (m + 1)],
                                rhs=hid_sb[:, kc, 512 * nh : 512 * (nh + 1)],
                                start=(kc == 0),
                                stop=(kc == 3),
                                skip_group_check=True,
                            )
                    ob = otp.tile([128, L], f32, tag="ob")
                    nc.vector.tensor_add(out=ob, in0=ps, in1=x_sb[:, m, :])
                    nc.sync.dma_start(out=outd[128 * m : 128 * (m + 1), :], in_=ob)

            if loop_n:
                with tc.For_i(0, loop_n, 1):
                    _emit_body()
            else:
                _emit_body()

    _split_waits(nc)
    return nc


_NC = None


def _get_nc():
    global _NC
    if _NC is None:
        _install_compat()
        _NC = build_nc()
    return _NC


def _host_prep(x, gamma, beta, w_qkv, b_qkv, w_proj, b_proj):
    x = np.asarray(x, np.float32)
    gamma = np.asarray(gamma, np.float32)
    beta = np.asarray(beta, np.float32)
    w_qkv = np.asarray(w_qkv, np.float32)
    b_qkv = np.asarray(b_qkv, np.float32)
    w_proj = np.asarray(w_proj, np.float32)
    b_proj = np.asarray(b_proj, np.float32)

    s2 = 1.0 / np.sqrt(CH)  # attention scale applied to q AND k => s^2 on q
    Wg = w_qkv * gamma[None, :]
    bb = w_qkv @ beta + b_qkv
    Wg = Wg.copy()
    Wg[0:C] *= s2
    bb = bb.copy()
    bb[0:C] *= s2

    shared = {
        "wqk": np.ascontiguousarray(Wg[0 : 2 * C].T).astype(BF16),
        "wv": np.ascontiguousarray(Wg[2 * C : 3 * C].T).astype(BF16),
        "wp": np.ascontiguousarray(w_proj.T).astype(BF16),
        "bqk": np.ascontiguousarray(bb[0 : 2 * C].reshape(8, 128).T).astype(
            np.float32
        ),
        "bvb": np.broadcast_to(bb[2 * C : 3 * C].reshape(1, C), (128, C)).astype(
            BF16
        ),
        "bp": np.ascontiguousarray(b_proj.reshape(4, 128).T).astype(np.float32),
        "ind": (np.arange(128)[:, None] // 16 == np.arange(8)[None, :]).astype(
            np.float32
        ),
        "indT": (np.arange(128)[None, :] // 16 == np.arange(8)[:, None]).astype(
            np.float32
        ),
    }
    in_maps = []
    for b in range(N_CORES):
        m = dict(shared)
        m["x"] = np.ascontiguousarray(x[b].reshape(C, L))
        in_maps.append(m)
    return in_maps


def run_spmd(in_maps, trace=False):
    from concourse.bass_utils import run_bass_kernel_spmd

    nc = _get_nc()
    return run_bass_kernel_spmd(
        nc, in_maps, core_ids=list(range(N_CORES)), trace=trace
    )


def kernel(x, gamma, beta, w_qkv, b_qkv, w_proj, b_proj):
    _install_compat()
    in_maps = _host_prep(x, gamma, beta, w_qkv, b_qkv, w_proj, b_proj)
    res = run_spmd(in_maps, trace=False)
    out = np.stack(
        [res.results[c]["out"].reshape(C, 32, 32) for c in range(N_CORES)]
    ).astype(np.float32)
    return out



# revision 18
# speedup vs baseline: 1.4935x; 1.0362x over previous
"""AttentionBlock (GroupNorm -> qkv -> 8-head attention -> proj -> residual)
as a distributed Bass/Tile kernel on 8 TRN2 NeuronCores.

Sharding: pure data-parallel over batch B=8 -> one batch element per core,
zero collectives. Each core computes its whole attention block.

Per-core algorithm (C=512, L=1024, NH=8, ch=64, G=32 groups):
  - GroupNorm stats via bn_stats per channel + tiny PE matmuls to reduce
    channel stats to group stats (16 channels/group) and broadcast back.
    gamma/beta are folded into the qkv weights host-side, attention scale
    (ch^-1/4 on q and k) is folded into the q weights as 1/sqrt(ch).
  - qkv as channel matmuls in bf16. q,k produced in natural [c, l] layout;
    v produced directly transposed ([l, c] layout) by swapping matmul
    operands, with the bias added via a K=1 ones-row matmul, so attention
    needs no on-chip transposes at all.
  - scores computed TRANSPOSED: sT[s, t] = k^T q (lhsT=k, rhs=q), softmax
    denominator via an extra ones-column appended to v^T (row 64 of the AV
    output accumulates sum_s P[s, t]).  exp on ScalarE from PSUM -> bf16.
  - AV: a[c, t] = (vT|1)^T @ P accumulated over 8 s-chunks.
  - 1/D via DVE reciprocal_approx_fast, broadcast across partitions with a
    DRAM-bounce DMA, applied while copying AV out of PSUM.
  - proj matmul in bf16 + residual add in f32.
"""

import sys
import types

import numpy as np
import ml_dtypes

BF16 = ml_dtypes.bfloat16

C = 512
L = 1024
NH = 8
CH = 64
G = 32
EPS = 1e-5
N_CORES = 8


# ---------------------------------------------------------------------------
# Environment compat (inlined so kernel.py is self-contained)
# ---------------------------------------------------------------------------
def _install_compat():
    # 1) NTFF profiling hook shim (image's antenv stub lacks axon_hooks).
    try:
        from antenv.axon_hooks import get_axon_ntff_profile_hook  # noqa: F401
    except ImportError:
        try:
            import antenv
            from trn_agent_boot.trn_boot import _ntff_profile_via_ctypes

            m = types.ModuleType("antenv.axon_hooks")
            m._hook = None
            m.set_axon_ntff_profile_hook = lambda h: setattr(m, "_hook", h)
            m.get_axon_ntff_profile_hook = lambda: m._hook
            sys.modules["antenv.axon_hooks"] = m
            antenv.axon_hooks = m
            m.set_axon_ntff_profile_hook(
                _ntff_profile_via_ctypes("/opt/axon/libaxon_pjrt.so")
            )
        except Exception:
            pass

    # 2) gpsimd.sem_clear over a wide semaphore range exceeds this walrus
    #    build's ISA payload limit ("ISA wrong length"); chunk the clears.
    import concourse.bass as bass

    if not getattr(bass.Bass.clear_and_free_semaphores, "_chunk_patch", False):
        _orig_clear = bass.Bass.clear_and_free_semaphores

        def _chunked_clear(self, sems, _orig=_orig_clear):
            sems = list(sems)
            for i in range(0, len(sems), 4):
                _orig(self, sems[i : i + 4])

        _chunked_clear._chunk_patch = True
        bass.Bass.clear_and_free_semaphores = _chunked_clear



def _split_waits(nc):
    """This walrus build accepts at most ONE semaphore wait per instruction;
    Tile emits up to 2 (and the closing drain more). Split the extras into
    standalone EVENT_SEM instructions inserted just before, on the same
    engine, which is semantically identical (same-engine program order)."""
    from concourse import mybir

    nid = 0
    for blk in nc.m.functions[0].blocks:
        new_list = []
        for inst in blk.instructions:
            si = inst.sync_info
            if si and si.on_wait and len(si.on_wait) > 1:
                waits = list(si.on_wait)
                si.on_wait = waits[-1:]
                for w in waits[:-1]:
                    nid += 1
                    ev = mybir.InstEventSemaphore(
                        name=f"WSPLIT-{nid}", ins=[], outs=[]
                    )
                    ev.engine = inst.engine
                    ev.sync_info = mybir.SyncInfo(on_wait=[w], on_update=[])
                    nc.register_instruction(ev, overwrite=True)
                    new_list.append(ev)
            new_list.append(inst)
        blk.instructions[:] = new_list


# ---------------------------------------------------------------------------
# Bass graph
# ---------------------------------------------------------------------------
def build_nc(loop_n=None):
    import concourse.bass as bass
    import concourse.tile as tile
    from concourse import mybir

    f32 = mybir.dt.float32
    bf = mybir.dt.bfloat16
    AF = mybir.ActivationFunctionType
    OP = mybir.AluOpType

    nc = bass.Bass(trn_type="TRN2")
    xd = nc.declare_dram_parameter("x", [C, L], f32, isOutput=False)
    wqkd = nc.declare_dram_parameter("wqk", [C, 2 * C], bf, isOutput=False)
    wvd = nc.declare_dram_parameter("wv", [C, C], bf, isOutput=False)
    wpd = nc.declare_dram_parameter("wp", [C, C], bf, isOutput=False)
    bqkd = nc.declare_dram_parameter("bqk", [128, 8], f32, isOutput=False)
    bvd = nc.declare_dram_parameter("bvb", [128, C], bf, isOutput=False)
    bpd = nc.declare_dram_parameter("bp", [128, 4], f32, isOutput=False)
    indd = nc.declare_dram_parameter("ind", [128, 8], f32, isOutput=False)
    indTd = nc.declare_dram_parameter("indT", [8, 128], f32, isOutput=False)
    outd = nc.declare_dram_parameter("out", [C, L], f32, isOutput=True)

    with tile.TileContext(nc) as tc:
        with (
            tc.tile_pool(name="cst", bufs=1) as cst,
            tc.tile_pool(name="act", bufs=1) as actp,
            tc.tile_pool(name="ptp", bufs=4) as ptp,
            tc.tile_pool(name="dnp", bufs=2) as dnp,
            tc.tile_pool(name="otp", bufs=2) as otp,
            tc.tile_pool(name="psp", bufs=1, space="PSUM") as psp,
            tc.tile_pool(name="drp", bufs=2, space="DRAM") as drp,
        ):
            # ---- weight/constant tiles (DMAs emitted inside the body AFTER
            # the x loads, so GroupNorm isn't queued behind 3MB of weights)
            wqk_sb = cst.tile([128, 4, 2 * C], bf)
            wv_sb = cst.tile([128, 4, C], bf)
            wp_sb = cst.tile([128, 4, C], bf)
            bqk_sb = cst.tile([128, 8], f32)
            bvb_sb = cst.tile([128, C], bf)
            bp_sb = cst.tile([128, 4], f32)
            ind_sb = cst.tile([128, 8], f32)
            indT_sb = cst.tile([8, 128], f32)
            eps8 = cst.tile([8, 1], f32)
            nc.vector.memset(eps8, EPS)

            def _emit_body():
                import concourse.bass as bass_mod

                x_sb = actp.tile([128, 4, L], f32)
                xr = xd[:, :].rearrange("(a p) o -> p a o", p=128)
                for t in range(4):
                    nc.sync.dma_start(out=x_sb[:, t, :], in_=xr[:, t, :])

                # constants, smallest/soonest-needed first
                nc.sync.dma_start(out=ind_sb, in_=indd[:, :])
                nc.sync.dma_start(out=indT_sb, in_=indTd[:, :])
                nc.sync.dma_start(out=bqk_sb, in_=bqkd[:, :])
                nc.sync.dma_start(
                    out=wqk_sb, in_=wqkd[:, :].rearrange("(a p) o -> p a o", p=128)
                )
                nc.sync.dma_start(
                    out=wv_sb, in_=wvd[:, :].rearrange("(a p) o -> p a o", p=128)
                )
                nc.sync.dma_start(out=bvb_sb, in_=bvd[:, :])
                nc.sync.dma_start(
                    out=wp_sb, in_=wpd[:, :].rearrange("(a p) o -> p a o", p=128)
                )
                nc.sync.dma_start(out=bp_sb, in_=bpd[:, :])

                xn_sb = actp.tile([128, 4, L], bf)
                q_sb = actp.tile([128, 4, L], bf)
                k_sb = actp.tile([128, 4, L], bf)
                vT_sb = actp.tile([128, 8, NH, CH + 1], bf)
                hid_sb = actp.tile([128, 4, L], bf)
                sc_sb = actp.tile([128, 4, 2], f32)

                # ---- GroupNorm statistics
                st6 = actp.tile([128, 4, 2, 6], f32)
                mv = actp.tile([128, 4, 2], f32)
                stats4 = actp.tile([128, 8], f32)
                for t in range(4):
                    for s in range(2):
                        nc.vector.bn_stats(
                            out=st6[:, t, s, :], in_=x_sb[:, t, 512 * s : 512 * (s + 1)]
                        )
                    nc.vector.bn_aggr(out=mv[:, t, :], in_=st6[:, t, :, :])
                    nc.vector.tensor_copy(
                        out=stats4[:, 2 * t : 2 * t + 1], in_=mv[:, t, 0:1]
                    )
                    nc.vector.tensor_mul(
                        out=stats4[:, 2 * t + 1 : 2 * t + 2],
                        in0=mv[:, t, 0:1],
                        in1=mv[:, t, 0:1],
                    )
                    nc.vector.tensor_add(
                        out=stats4[:, 2 * t + 1 : 2 * t + 2],
                        in0=stats4[:, 2 * t + 1 : 2 * t + 2],
                        in1=mv[:, t, 1:2],
                    )
                gmm = psp.tile([8, 8], f32, tag="B", bufs=2)
                nc.tensor.matmul(gmm, lhsT=ind_sb, rhs=stats4, start=True, stop=True)
                gm = actp.tile([8, 8], f32)
                nc.vector.tensor_scalar_mul(out=gm, in0=gmm, scalar1=1.0 / 16.0)
                gmr = gm.rearrange("g (t s) -> g t s", s=2)
                msq = actp.tile([8, 4], f32)
                nc.vector.tensor_mul(out=msq, in0=gmr[:, :, 0], in1=gmr[:, :, 0])
                gv = actp.tile([8, 4], f32)
                nc.vector.tensor_tensor(
                    out=gv, in0=gmr[:, :, 1], in1=msq, op=OP.subtract
                )
                # rsqrt(v + eps) = exp(-0.5 * ln(v + eps)): Log and Exp share one
                # ACT table set (natural_log_exp_and_others), so the softmax Exp
                # later needs no table switch.
                sd = actp.tile([8, 4], f32)
                nc.scalar.activation(out=sd, in_=gv, func=AF.Ln, bias=eps8, scale=1.0)
                inv8 = actp.tile([8, 4], f32)
                nc.scalar.activation(out=inv8, in_=sd, func=AF.Exp, scale=-0.5)
                sh8 = actp.tile([8, 4], f32)
                nc.vector.tensor_mul(out=sh8, in0=gmr[:, :, 0], in1=inv8)
                nc.vector.tensor_scalar_mul(out=sh8, in0=sh8, scalar1=-1.0)
                gs = actp.tile([8, 8], f32)
                gsr = gs.rearrange("g (t s) -> g t s", s=2)
                nc.vector.tensor_copy(out=gsr[:, :, 0], in_=inv8)
                nc.vector.tensor_copy(out=gsr[:, :, 1], in_=sh8)
                nb = psp.tile([128, 8], f32, tag="B", bufs=2)
                nc.tensor.matmul(nb, lhsT=indT_sb, rhs=gs, start=True, stop=True)
                nc.vector.tensor_copy(
                    out=sc_sb.rearrange("p t s -> p (t s)"), in_=nb
                )
                for t in range(4):
                    nc.vector.tensor_scalar(
                        out=xn_sb[:, t, :],
                        in0=x_sb[:, t, :],
                        scalar1=sc_sb[:, t, 0:1],
                        scalar2=sc_sb[:, t, 1:2],
                        op0=OP.mult,
                        op1=OP.add,
                    )

                # ---- emission helpers for the software pipeline ----------
                def emit_qk_tile(m):
                    # one m-tile of the q/k matmul, kc-outer so each wqk
                    # stationary block is loaded once for both nh halves;
                    # shares tag-A PSUM slots with the score tiles
                    ps = psp.tile([128, L], f32, tag="A", bufs=2, name=f"qkv{m}")
                    for kc in range(4):
                        for nh in range(2):
                            nc.tensor.matmul(
                                ps[:, 512 * nh : 512 * (nh + 1)],
                                lhsT=wqk_sb[:, kc, 128 * m : 128 * (m + 1)],
                                rhs=xn_sb[:, kc, 512 * nh : 512 * (nh + 1)],
                                start=(kc == 0),
                                stop=(kc == 3),
                                skip_group_check=True,
                            )
                    dst = q_sb if m < 4 else k_sb
                    nc.vector.tensor_scalar_add(
                        out=dst[:, m % 4, :], in0=ps, scalar1=bqk_sb[:, m : m + 1]
                    )

                def emit_v_chunk(lt):
                    # vT[l, c] for one 128-l chunk (transposed v via swapped
                    # matmul operands)
                    ps = psp.tile([128, C], f32, tag="A", bufs=2, name=f"v{lt}")
                    for kc in range(4):
                        nc.tensor.matmul(
                            ps,
                            lhsT=xn_sb[:, kc, 128 * lt : 128 * (lt + 1)],
                            rhs=wv_sb[:, kc, :],
                            start=(kc == 0),
                            stop=(kc == 3),
                            skip_group_check=True,
                        )
                    nc.vector.tensor_tensor(
                        out=vT_sb[:, lt, :, 0:CH],
                        in0=ps.rearrange("p (h c) -> p h c", h=NH),
                        in1=bvb_sb.rearrange("p (h c) -> p h c", h=NH),
                        op=OP.add,
                    )

                pts_t = {}
                av_t = {}

                def emit_qk_chunk(j, st):
                    # scores^T [s-chunk, t] for head pair j + exp on ScalarE
                    pss = [
                        psp.tile([128, L], f32, tag="A", bufs=2, name=f"qkt{j}_{st}_0"),
                        psp.tile([128, L], f32, tag="A", bufs=2, name=f"qkt{j}_{st}_1"),
                    ]
                    for hh in range(2):
                        po = 64 * hh
                        for nh in range(2):
                            nc.tensor.matmul(
                                pss[hh][:, 512 * nh : 512 * (nh + 1)],
                                lhsT=k_sb[po : po + 64, j, 128 * st : 128 * (st + 1)],
                                rhs=q_sb[po : po + 64, j, 512 * nh : 512 * (nh + 1)],
                                start=True,
                                stop=True,
                                skip_group_check=True,
                            )
                    for hh in range(2):
                        nc.scalar.activation(
                            out=pts_t[j][hh][:, st, :], in_=pss[hh], func=AF.Exp
                        )

                def emit_av_chunk(j, st):
                    # accumulate (vT|1)^T @ P for both heads of pair j
                    for hh in range(2):
                        for nh in range(2):
                            nc.tensor.matmul(
                                av_t[j][hh][:, 512 * nh : 512 * (nh + 1)],
                                lhsT=vT_sb[:, st, 2 * j + hh, :],
                                rhs=pts_t[j][hh][:, st, 512 * nh : 512 * (nh + 1)],
                                start=(st == 0),
                                stop=(st == 7),
                                skip_group_check=True,
                            )

                def emit_norm(j):
                    # Softmax normalization. First copy each head's AV (incl.
                    # the D row) out of PSUM into bf16 SBUF — this frees the
                    # PSUM B slots immediately so the next pair's AV can
                    # accumulate while the 1/D chain below is in flight.
                    ah = []
                    for hh in range(2):
                        a = dnp.tile([CH + 1, L], bf, tag=f"ah{hh}")
                        nc.vector.tensor_copy(out=a, in_=av_t[j][hh])
                        ah.append(a)
                    # DVE's iterative-divide reciprocal costs ~6 cyc/elem PER
                    # LANE (free-size bound): [1, 1024] would be 6.5us. Retile
                    # both D rows to [16, 128] via SBUF->SBUF DMA so 16 lanes
                    # share the work (~0.8us). NB: keep the source partition
                    # dim at 1 — splitting the free dim into the AP's
                    # partition slot would read physical partitions 1..7.
                    dT = dnp.tile([16, 128], bf, tag="dT")
                    for hh in range(2):
                        nc.sync.dma_start(
                            out=dT[8 * hh : 8 * (hh + 1), :],
                            in_=ah[hh][CH : CH + 1, :].rearrange(
                                "o (p a) -> o p a", p=8
                            ),
                        )
                    rT = dnp.tile([16, 128], bf, tag="rT")
                    with nc.allow_low_precision(
                        reason="softmax 1/D in bf16 is within tolerance"
                    ):
                        nc.vector.reciprocal(out=rT, in_=dT)
                    ddr = drp.tile([2, L], bf, tag="ddr")
                    nc.sync.dma_start(
                        out=ddr[:, :].rearrange("h (p a) -> (h p) a", p=8),
                        in_=rT,
                    )
                    for hh in (1, 0):
                        dbb = dnp.tile([CH, L], bf, tag=f"dbb{hh}")
                        row = ddr[hh : hh + 1, :]
                        bcast = bass_mod.AP(
                            tensor=row.tensor,
                            offset=row.offset,
                            ap=[[0, CH]] + list(row.ap[1:]),
                        )
                        nc.sync.dma_start(out=dbb, in_=bcast)
                        if hh == 0:
                            nc.vector.tensor_mul(
                                out=hid_sb[0:CH, j, :], in0=ah[0][0:CH, :], in1=dbb
                            )
                        else:
                            tmpo = dnp.tile([CH, L], bf, tag="tmpo")
                            nc.vector.tensor_mul(
                                out=tmpo, in0=ah[1][0:CH, :], in1=dbb
                            )
                            nc.sync.dma_start(out=hid_sb[CH:128, j, :], in_=tmpo)

                def alloc_pts(j):
                    pts_t[j] = [
                        ptp.tile([128, 8, L], bf, tag="pt", name=f"pt{j}_0"),
                        ptp.tile([128, 8, L], bf, tag="pt", name=f"pt{j}_1"),
                    ]

                def alloc_av(j):
                    av_t[j] = [
                        psp.tile([CH + 1, L], f32, tag="B", bufs=2, name=f"av{j}_0"),
                        psp.tile([CH + 1, L], f32, tag="B", bufs=2, name=f"av{j}_1"),
                    ]

                # ---- software pipeline --------------------------------------
                # PE order: q/k for pair 0 first so exp (the ScalarE
                # bottleneck) starts ASAP; v chunks + later q/k pairs + the
                # previous pair's AV fill PE slack inside each st loop, so
                # ScalarE never starves and PE never idles long.
                nc.vector.memset(vT_sb[:, :, :, CH : CH + 1], 1.0)
                emit_qk_tile(0)
                emit_qk_tile(4)

                # fillers[j][st] -> list of zero-arg emitters
                def qkt(m):
                    return lambda: emit_qk_tile(m)

                def vch(lt):
                    return lambda: emit_v_chunk(lt)

                # v chunk lt must be emitted before AV(j=0, lt); q/k tiles
                # for pair j+1 must be emitted before QK(j+1, 0)
                fillers = {
                    0: {
                        0: [vch(0)], 1: [vch(1)], 2: [vch(2)], 3: [vch(3)],
                        4: [vch(4)], 5: [vch(5), qkt(1)],
                        6: [vch(6), qkt(5)], 7: [vch(7)],
                    },
                    1: {4: [qkt(2)], 6: [qkt(6)]},
                    2: {4: [qkt(3)], 6: [qkt(7)]},
                    3: {},
                }

                for j in range(4):
                    alloc_pts(j)
                    alloc_av(j)
                    for st in range(8):
                        emit_qk_chunk(j, st)
                        if st > 0:
                            emit_av_chunk(j, st - 1)
                        for f in fillers[j].get(st, []):
                            f()
                    emit_av_chunk(j, 7)
                    emit_norm(j)
                    if j == 0:
                        # residual base: x + b_proj (in place), off the
                        # critical path while ScalarE grinds exps
                        for m in range(4):
                            nc.vector.tensor_scalar_add(
                                out=x_sb[:, m, :],
                                in0=x_sb[:, m, :],
                                scalar1=bp_sb[:, m : m + 1],
                            )

                # ---- proj + residual, kc-outer across all 4 m-tiles: the
                # kc<3 partials only need hid pairs 0..2 and run during the
                # last norm chain; only the kc=3 matmuls wait on hid[:, 3].
                # m2/m3 live in tag-B PSUM slots freed by the norm(3) copies.
                pst = []
                for m in range(4):
                    pst.append(
                        psp.tile(
                            [128, L], f32, tag=("A" if m < 2 else "B"),
                            bufs=2, name=f"proj{m}",
                        )
                    )
                for kc in range(4):
                    for m in range(4):
                        for nh in range(2):
                            nc.tensor.matmul(
                                pst[m][:, 512 * nh : 512 * (nh + 1)],
                                lhsT=wp_sb[:, kc, 128 * m : 128 * (m + 1)],
                                rhs=hid_sb[:, kc, 512 * nh : 512 * (nh + 1)],
                                start=(kc == 0),
                                stop=(kc == 3),
                                skip_group_check=True,
                            )
                for m in range(4):
                    ob = otp.tile([128, L], f32, tag="ob")
                    nc.vector.tensor_add(out=ob, in0=pst[m], in1=x_sb[:, m, :])
                    nc.sync.dma_start(out=outd[128 * m : 128 * (m + 1), :], in_=ob)

            if loop_n:
                with tc.For_i(0, loop_n, 1):
                    _emit_body()
            else:
                _emit_body()

    _split_waits(nc)
    return nc


_NC = None


def _get_nc():
    global _NC
    if _NC is None:
        _install_compat()
        _NC = build_nc()
    return _NC


def _host_prep(x, gamma, beta, w_qkv, b_qkv, w_proj, b_proj):
    x = np.asarray(x, np.float32)
    gamma = np.asarray(gamma, np.float32)
    beta = np.asarray(beta, np.float32)
    w_qkv = np.asarray(w_qkv, np.float32)
    b_qkv = np.asarray(b_qkv, np.float32)
    w_proj = np.asarray(w_proj, np.float32)
    b_proj = np.asarray(b_proj, np.float32)

    s2 = 1.0 / np.sqrt(CH)  # attention scale applied to q AND k => s^2 on q
    Wg = w_qkv * gamma[None, :]
    bb = w_qkv @ beta + b_qkv
    Wg = Wg.copy()
    Wg[0:C] *= s2
    bb = bb.copy()
    bb[0:C] *= s2

    shared = {
        "wqk": np.ascontiguousarray(Wg[0 : 2 * C].T).astype(BF16),
        "wv": np.ascontiguousarray(Wg[2 * C : 3 * C].T).astype(BF16),
        "wp": np.ascontiguousarray(w_proj.T).astype(BF16),
        "bqk": np.ascontiguousarray(bb[0 : 2 * C].reshape(8, 128).T).astype(
            np.float32
        ),
        "bvb": np.broadcast_to(bb[2 * C : 3 * C].reshape(1, C), (128, C)).astype(
            BF16
        ),
        "bp": np.ascontiguousarray(b_proj.reshape(4, 128).T).astype(np.float32),
        "ind": (np.arange(128)[:, None] // 16 == np.arange(8)[None, :]).astype(
            np.float32
        ),
        "indT": (np.arange(128)[None, :] // 16 == np.arange(8)[:, None]).astype(
            np.float32
        ),
    }
    in_maps = []
    for b in range(N_CORES):
        m = dict(shared)
        m["x"] = np.ascontiguousarray(x[b].reshape(C, L))
        in_maps.append(m)
    return in_maps


def run_spmd(in_maps, trace=False):
    from concourse.bass_utils import run_bass_kernel_spmd

    nc = _get_nc()
    return run_bass_kernel_spmd(
        nc, in_maps, core_ids=list(range(N_CORES)), trace=trace
    )


def kernel(x, gamma, beta, w_qkv, b_qkv, w_proj, b_proj):
    _install_compat()
    in_maps = _host_prep(x, gamma, beta, w_qkv, b_qkv, w_proj, b_proj)
    res = run_spmd(in_maps, trace=False)
    out = np.stack(
        [res.results[c]["out"].reshape(C, 32, 32) for c in range(N_CORES)]
    ).astype(np.float32)
    return out

